# revision 6
# baseline (speedup 1.0000x reference)
"""Batch depthwise conv2d (per-sample 3x3 filters) on 8 TRN2 NeuronCores.

x: [32, 128, 128, 128] (b, h, w, ci) fp32
f: [32, 3, 3, 128, 1]  (b, kh, kw, ci, 1) fp32
out: [32, 126, 126, 128] VALID, stride 1.

Data parallel: 4 samples per core. Per sample on each core:
  1. x loaded naturally [h, (w c)] into SBUF in w-blocks.
  2. PE transposes [h, c] slices (fixed w) -> x_t [c, s] with s = w*128 + h
     (channel-major; all 9 tap shifts become free-dim offsets dw*128+dh).
  3. 9 matmuls per output chunk: lhsT = diag(f[dh,dw,:]), rhs = x_t shifted,
     accumulated in PSUM [c, s_chunk] (fp32).
  4. PE transposes back per w-col [c, h] -> [h, c], assembled into [y, (x c)]
     tiles and DMA'd out.

Compute dtype for the tap matmuls is switchable: float32r (full PE rate at
N=512, reduced-precision multiply), bfloat16 (full rate, casts on load), or
float32 (exact, 4x slower).
"""

import os
from contextlib import ExitStack

import numpy as np

import concourse.bacc as bacc
import concourse.bass as bass
import concourse.mybir as mybir
import concourse.tile as tile
from concourse.bass_utils import run_bass_kernel_spmd

N_CORES = 8
B_PER = 4
H = W = C = 128
HO = WO = 126
FULL_B = 32

# tap compute mode: "f32r" | "bf16" | "f32"
KMODE = os.environ.get("BASS_KMODE", "f32r")

F32 = mybir.dt.float32
F32R = mybir.dt.float32r
BF16 = mybir.dt.bfloat16

# storage dtype for x path (DMA + transposes)
SDT = BF16 if KMODE == "bf16" else F32
# dtype of x_t and diag (the tap matmul operands); fp32r producers must
# write fp32r directly (walrus requires rounded-to-fp32r inputs).
XT_DT = {"f32r": F32R, "bf16": BF16, "f32": F32}[KMODE]

S_TOT = W * H          # 16384 flattened spatial (w-major)
S_PAD = S_TOT + 384    # room for tap offsets up to 2*128+2 past valid reads
CHUNK = 512            # psum free-dim per tap accumulation group
NCHUNK = S_TOT // CHUNK  # 32
CG = 3                 # chunks per tap group (shares diag weight loads)
WBLK = 32              # w-cols per input DMA


def _emit(ctx: ExitStack, tc: tile.TileContext):
    nc = tc.nc
    x = nc.dram_tensor("x", [B_PER, H, W, C], F32, kind="ExternalInput").ap()
    f = nc.dram_tensor("f", [B_PER, 3, 3, C, 1], F32, kind="ExternalInput").ap()
    y = nc.dram_tensor("y", [B_PER, HO, WO, C], F32, kind="ExternalOutput").ap()

    xin = ctx.enter_context(tc.tile_pool(name="xin", bufs=3))
    xtp = ctx.enter_context(tc.tile_pool(name="xtp", bufs=1))
    qp = ctx.enter_context(tc.tile_pool(name="qp", bufs=4))
    outp = ctx.enter_context(tc.tile_pool(name="outp", bufs=2))
    fpool = ctx.enter_context(tc.tile_pool(name="fpool", bufs=2))
    consts = ctx.enter_context(tc.tile_pool(name="consts", bufs=1))
    ps_in = ctx.enter_context(tc.tile_pool(name="ps_in", bufs=2, space="PSUM"))
    ps_tap = ctx.enter_context(tc.tile_pool(name="ps_tap", bufs=CG, space="PSUM"))
    ps_out = ctx.enter_context(tc.tile_pool(name="ps_out", bufs=1, space="PSUM"))

    # Identity matrices. Built on gpsimd, then re-produced by ACT so that
    # every PE-side input has ACT as its single producer engine (each PE
    # transpose instruction has exactly ONE hardware sync-wait slot).
    ident_g = consts.tile([128, 128], F32)
    nc.gpsimd.memset(ident_g[:], 1.0)
    nc.gpsimd.affine_select(
        ident_g[:], ident_g[:], pattern=[[-1, 128]],
        compare_op=mybir.AluOpType.is_equal, fill=0.0, base=0,
        channel_multiplier=1,
    )
    ident_f32 = consts.tile([128, 128], F32)
    nc.scalar.copy(ident_f32[:], ident_g[:])
    if SDT == F32:
        ident_s = ident_f32
    else:
        ident_s = consts.tile([128, 128], SDT)
        nc.scalar.copy(ident_s[:], ident_g[:])

    # Dummy transpose: absorbs the one-time ACT(ident) wait so the first
    # real input transpose only needs its DMA wait.
    warm = ps_in.tile([C, CHUNK], F32, name="tp", tag="tp")
    nc.tensor.transpose(warm[:, 0:128], ident_f32[:], ident_f32[:])

    for b in range(B_PER):
        # ---- filters: strided DMA -> [c, 9] columns -> 9 diag matrices ----
        fcols = fpool.tile([C, 9], F32)
        nc.gpsimd.dma_start(fcols[:], f[b].rearrange("kh kw c m -> c (kh kw m)"))
        diag = fpool.tile([C, 9 * C], XT_DT)
        for r in range(9):
            nc.vector.tensor_scalar_mul(
                diag[:, r * C:(r + 1) * C], ident_s[:], fcols[:, r:r + 1]
            )

        # ---- load x + transpose to channel-major x_t[c, w*128+h] ----
        x_t = xtp.tile([C, S_PAD], XT_DT)
        for wb in range(W // WBLK):
            xs = xin.tile([H, WBLK * C], SDT)
            src = x[b, :, wb * WBLK:(wb + 1) * WBLK, :].rearrange("h w c -> h (w c)")
            if SDT == F32:
                nc.sync.dma_start(xs[:], src)
            else:
                nc.gpsimd.dma_start(xs[:], src)  # SWDGE cast fp32->bf16
            for g in range(WBLK // 4):
                # group 0 of each block waits on the block DMA; its PSUM
                # slot comes from a dedicated tag whose release is ancient,
                # so the transpose carries only the DMA wait.
                tag = "tpf" if g == 0 else "tp"
                tp = ps_in.tile([C, 4 * H], SDT, name="tp", tag=tag)
                for j in range(4):
                    wl = g * 4 + j
                    nc.tensor.transpose(
                        tp[:, j * H:(j + 1) * H],
                        xs[:, wl * C:(wl + 1) * C],
                        ident_s[:],
                    )
                wg = wb * WBLK + g * 4
                nc.scalar.copy(x_t[:, wg * H:(wg + 4) * H], tp[:])

        # ---- 9-tap diag matmuls, PSUM accumulate, chunk groups of CG ----
        qs = []
        cg0 = 0
        while cg0 < NCHUNK:
            ncg = min(CG, NCHUNK - cg0)
            psums = [
                ps_tap.tile([C, CHUNK], F32, name="tap_ps", tag="tap_ps")
                for _ in range(ncg)
            ]
            ti = 0
            for dw in range(3):
                for dh in range(3):
                    off = dw * H + dh
                    r = dh * 3 + dw  # f row index (kh=dh, kw=dw)
                    dg = diag[:, r * C:(r + 1) * C]
                    for j in range(ncg):
                        s0 = (cg0 + j) * CHUNK
                        nc.tensor.matmul(
                            psums[j][:],
                            dg,
                            x_t[:, s0 + off: s0 + off + CHUNK],
                            start=(ti == 0),
                            stop=(ti == 8),
                        )
                    ti += 1
            for j in range(ncg):
                q = qp.tile([C, CHUNK], F32)
                nc.vector.tensor_copy(q[:], psums[j][:])
                qs.append(q)
            cg0 += ncg

        # ---- transpose back [c, h] -> [h, c] per valid w-col; DMA out ----
        for ob in range(8):  # output blocks of 16 w-cols
            w0 = ob * 16
            nw = min(16, WO - w0)  # 16,...,16,14
            osb = outp.tile([128, 16 * C], F32)
            for g in range(4):  # 4 chunks of 4 w-cols each
                chunk_idx = ob * 4 + g
                nv = max(0, min(4, WO - (w0 + g * 4)))
                if nv == 0:
                    continue
                ops_t = ps_out.tile([128, 4 * C], F32, name="ops_t", tag="ops_t")
                q = qs[chunk_idx]
                for j in range(nv):
                    nc.tensor.transpose(
                        ops_t[:, j * C:(j + 1) * C],
                        q[:, j * H:(j + 1) * H],
                        ident_f32[:],
                    )
                nc.vector.tensor_copy(
                    osb[0:HO, g * 4 * C:(g * 4 + nv) * C], ops_t[0:HO, 0:nv * C]
                )
            dst = y[b, :, w0:w0 + nw, :].rearrange("h w c -> h (w c)")
            nc.sync.dma_start(dst, osb[0:HO, 0:nw * C])


_NC_CACHE = {}


def _build():
    key = KMODE
    if key not in _NC_CACHE:
        nc = bacc.Bacc("TRN2", target_bir_lowering=False, debug=False)
        with tile.TileContext(nc) as tc, ExitStack() as ctx:
            _emit(ctx, tc)
        nc.compile()
        _NC_CACHE[key] = nc
    return _NC_CACHE[key]


def kernel(x: np.ndarray, f: np.ndarray) -> np.ndarray:
    assert x.shape == (FULL_B, H, W, C) and f.shape == (FULL_B, 3, 3, C, 1)
    x = np.ascontiguousarray(x, dtype=np.float32)
    f = np.ascontiguousarray(f, dtype=np.float32)
    nc = _build()
    in_maps = [
        {
            "x": x[i * B_PER:(i + 1) * B_PER],
            "f": f[i * B_PER:(i + 1) * B_PER],
        }
        for i in range(N_CORES)
    ]
    res = run_bass_kernel_spmd(nc, in_maps, core_ids=list(range(N_CORES)))
    return np.concatenate([r["y"] for r in res.results], axis=0)


if __name__ == "__main__":
    xs = np.random.randn(FULL_B, H, W, C).astype(np.float32)
    fs = np.random.randn(FULL_B, 3, 3, C, 1).astype(np.float32)
    out = kernel(xs, fs)
    print(out.shape, out.dtype)


# revision 9
# speedup vs baseline: 109.6268x; 109.6268x over previous
"""Batch depthwise conv2d (per-sample 3x3 filters) on 8 TRN2 NeuronCores.

x: [32, 128, 128, 128] (b, h, w, ci) fp32
f: [32, 3, 3, 128, 1]  (b, kh, kw, ci, 1) fp32
out: [32, 126, 126, 128] VALID, stride 1.

Data parallel: 4 samples per core. Per sample on each core:
  1. x loaded naturally [h, (w c)] into SBUF in w-blocks.
  2. PE transposes [h, c] slices (fixed w) -> x_t [c, s] with s = w*128 + h
     (channel-major; all 9 tap shifts become free-dim offsets dw*128+dh).
  3. 9 matmuls per output chunk: lhsT = diag(f[dh,dw,:]), rhs = x_t shifted,
     accumulated in PSUM [c, s_chunk] (fp32).
  4. PE transposes back per w-col [c, h] -> [h, c], assembled into [y, (x c)]
     tiles and DMA'd out.

Compute dtype for the tap matmuls is switchable: float32r (full PE rate at
N=512, reduced-precision multiply), bfloat16 (full rate, casts on load), or
float32 (exact, 4x slower).
"""

import os
from contextlib import ExitStack

import numpy as np

import concourse.bacc as bacc
import concourse.bass as bass
import concourse.mybir as mybir
import concourse.tile as tile
from concourse.bass_utils import run_bass_kernel_spmd

N_CORES = 8
B_PER = 4
H = W = C = 128
HO = WO = 126
FULL_B = 32

# tap compute mode: "f32r" | "bf16" | "f32"
KMODE = os.environ.get("BASS_KMODE", "f32r")
# benchmark-only: repeat the whole body LOOP_K times in a hardware loop
LOOP_K = int(os.environ.get("BASS_LOOP_K", "1"))

F32 = mybir.dt.float32
F32R = mybir.dt.float32r
BF16 = mybir.dt.bfloat16

# storage dtype for x path (DMA + transposes)
SDT = BF16 if KMODE == "bf16" else F32
# dtype of x_t and diag (the tap matmul operands); fp32r producers must
# write fp32r directly (walrus requires rounded-to-fp32r inputs).
XT_DT = {"f32r": F32R, "bf16": BF16, "f32": F32}[KMODE]

S_TOT = W * H          # 16384 flattened spatial (w-major)
S_PAD = S_TOT + 384    # room for tap offsets up to 2*128+2 past valid reads
CHUNK = 512            # psum free-dim per tap accumulation group
NCHUNK = S_TOT // CHUNK  # 32
CG = 4                 # chunks per tap group (shares diag weight loads)
WBLK = 32              # w-cols per input DMA


def _emit(ctx: ExitStack, tc: tile.TileContext):
    nc = tc.nc
    x = nc.dram_tensor("x", [B_PER, H, W, C], F32, kind="ExternalInput").ap()
    f = nc.dram_tensor("f", [B_PER, 3, 3, C, 1], F32, kind="ExternalInput").ap()
    y = nc.dram_tensor("y", [B_PER, HO, WO, C], F32, kind="ExternalOutput").ap()

    xin = ctx.enter_context(tc.tile_pool(name="xin", bufs=3))
    xtp = ctx.enter_context(tc.tile_pool(name="xtp", bufs=1))
    qp = ctx.enter_context(tc.tile_pool(name="qp", bufs=4))
    outp = ctx.enter_context(tc.tile_pool(name="outp", bufs=2))
    fpool = ctx.enter_context(tc.tile_pool(name="fpool", bufs=2))
    consts = ctx.enter_context(tc.tile_pool(name="consts", bufs=1))
    ps_in = ctx.enter_context(tc.tile_pool(name="ps_in", bufs=2, space="PSUM"))
    ps_tap = ctx.enter_context(tc.tile_pool(name="ps_tap", bufs=CG, space="PSUM"))
    ps_out = ctx.enter_context(tc.tile_pool(name="ps_out", bufs=2, space="PSUM"))

    # Identity matrices. Built on gpsimd, then re-produced by ACT so that
    # every PE-side input has ACT as its single producer engine (each PE
    # transpose instruction has exactly ONE hardware sync-wait slot).
    ident_g = consts.tile([128, 128], F32)
    nc.gpsimd.memset(ident_g[:], 1.0)
    nc.gpsimd.affine_select(
        ident_g[:], ident_g[:], pattern=[[-1, 128]],
        compare_op=mybir.AluOpType.is_equal, fill=0.0, base=0,
        channel_multiplier=1,
    )
    ident_f32 = consts.tile([128, 128], F32)
    nc.scalar.copy(ident_f32[:], ident_g[:])
    if SDT == F32:
        ident_s = ident_f32
    else:
        ident_s = consts.tile([128, 128], SDT)
        nc.scalar.copy(ident_s[:], ident_g[:])

    # Dummy transpose: absorbs the one-time ACT(ident) wait so the first
    # real input transpose only needs its DMA wait.
    warm = ps_in.tile([C, CHUNK], F32, name="tp", tag="tp")
    nc.tensor.transpose(warm[:, 0:128], ident_f32[:], ident_f32[:])

    if LOOP_K > 1:
        loop_cm = tc.For_i(0, LOOP_K, 1)
        loop_cm.__enter__()

    for b in range(B_PER):
        # ---- filters: strided DMA -> [c, 9] columns -> 9 diag matrices ----
        fcols = fpool.tile([C, 9], F32)
        nc.gpsimd.dma_start(fcols[:], f[b].rearrange("kh kw c m -> c (kh kw m)"))
        diag = fpool.tile([C, 9 * C], XT_DT)
        for r in range(9):
            nc.vector.tensor_scalar_mul(
                diag[:, r * C:(r + 1) * C], ident_s[:], fcols[:, r:r + 1]
            )

        # ---- load x + transpose to channel-major x_t[c, w*128+h] ----
        # Split into two halves so sample b+1's copies into half A overlap
        # sample b's tap matmuls on half B (x_t is single-buffered).
        # A holds w-cols 0..65 (tap chunks 0..15 read w<=65);
        # B holds w-cols 64..127 plus tail pad (chunks 16..31, local offset).
        xtA = xtp.tile([C, 66 * H + 384], XT_DT, name="xtA", tag="xtA")
        xtB = xtp.tile([C, 64 * H + 384], XT_DT, name="xtB", tag="xtB")
        for wb in range(W // WBLK):
            xs = xin.tile([H, WBLK * C], SDT)
            src = x[b, :, wb * WBLK:(wb + 1) * WBLK, :].rearrange("h w c -> h (w c)")
            if SDT == F32:
                nc.sync.dma_start(xs[:], src)
            else:
                nc.gpsimd.dma_start(xs[:], src)  # SWDGE cast fp32->bf16
            for g in range(WBLK // 4):
                tp = ps_in.tile([C, 4 * H], SDT, name="tp", tag="tp")
                for j in range(4):
                    wl = g * 4 + j
                    nc.tensor.transpose(
                        tp[:, j * H:(j + 1) * H],
                        xs[:, wl * C:(wl + 1) * C],
                        ident_s[:],
                    )
                wg = wb * WBLK + g * 4
                if wg + 4 <= 66:
                    nc.scalar.copy(xtA[:, wg * H:(wg + 4) * H], tp[:])
                elif wg >= 66:
                    nc.scalar.copy(xtB[:, (wg - 64) * H:(wg - 60) * H], tp[:])
                else:  # wg == 64: w64..67 straddles -> copy into both halves
                    nc.scalar.copy(xtA[:, 64 * H:66 * H], tp[:, 0:2 * H])
                    nc.scalar.copy(xtB[:, 0:4 * H], tp[:])

        # ---- 9-tap diag matmuls, PSUM accumulate, chunk groups of CG ----
        qs = []
        cg0 = 0
        while cg0 < NCHUNK:
            ncg = min(CG, NCHUNK - cg0)
            psums = [
                ps_tap.tile([C, CHUNK], F32, name="tap_ps", tag="tap_ps")
                for _ in range(ncg)
            ]
            ti = 0
            for dw in range(3):
                for dh in range(3):
                    off = dw * H + dh
                    r = dh * 3 + dw  # f row index (kh=dh, kw=dw)
                    dg = diag[:, r * C:(r + 1) * C]
                    for j in range(ncg):
                        s0 = (cg0 + j) * CHUNK
                        if cg0 + j < 16:
                            xsrc = xtA[:, s0 + off: s0 + off + CHUNK]
                        else:
                            sl = s0 - 64 * H
                            xsrc = xtB[:, sl + off: sl + off + CHUNK]
                        nc.tensor.matmul(
                            psums[j][:],
                            dg,
                            xsrc,
                            start=(ti == 0),
                            stop=(ti == 8),
                        )
                    ti += 1
            for j in range(ncg):
                q = qp.tile([C, CHUNK], F32)
                nc.vector.tensor_copy(q[:], psums[j][:])
                qs.append(q)
            cg0 += ncg

        # ---- transpose back [c, h] -> [h, c] per valid w-col; DMA out ----
        for ob in range(8):  # output blocks of 16 w-cols
            w0 = ob * 16
            nw = min(16, WO - w0)  # 16,...,16,14
            osb = outp.tile([128, 16 * C], F32)
            for g in range(4):  # 4 chunks of 4 w-cols each
                chunk_idx = ob * 4 + g
                nv = max(0, min(4, WO - (w0 + g * 4)))
                if nv == 0:
                    continue
                ops_t = ps_out.tile([128, 4 * C], F32, name="ops_t", tag="ops_t")
                q = qs[chunk_idx]
                for j in range(nv):
                    nc.tensor.transpose(
                        ops_t[:, j * C:(j + 1) * C],
                        q[:, j * H:(j + 1) * H],
                        ident_f32[:],
                    )
                nc.vector.tensor_copy(
                    osb[0:HO, g * 4 * C:(g * 4 + nv) * C], ops_t[0:HO, 0:nv * C]
                )
            dst = y[b, :, w0:w0 + nw, :].rearrange("h w c -> h (w c)")
            nc.sync.dma_start(dst, osb[0:HO, 0:nw * C])

    if LOOP_K > 1:
        loop_cm.__exit__(None, None, None)


_NC_CACHE = {}


def _build():
    key = (KMODE, LOOP_K)
    if key not in _NC_CACHE:
        nc = bacc.Bacc("TRN2", target_bir_lowering=False, debug=False)
        with tile.TileContext(nc) as tc, ExitStack() as ctx:
            _emit(ctx, tc)
        nc.compile()
        _NC_CACHE[key] = nc
    return _NC_CACHE[key]


def kernel(x: np.ndarray, f: np.ndarray) -> np.ndarray:
    assert x.shape == (FULL_B, H, W, C) and f.shape == (FULL_B, 3, 3, C, 1)
    x = np.ascontiguousarray(x, dtype=np.float32)
    f = np.ascontiguousarray(f, dtype=np.float32)
    nc = _build()
    in_maps = [
        {
            "x": x[i * B_PER:(i + 1) * B_PER],
            "f": f[i * B_PER:(i + 1) * B_PER],
        }
        for i in range(N_CORES)
    ]
    res = run_bass_kernel_spmd(nc, in_maps, core_ids=list(range(N_CORES)))
    return np.concatenate([r["y"] for r in res.results], axis=0)


if __name__ == "__main__":
    xs = np.random.randn(FULL_B, H, W, C).astype(np.float32)
    fs = np.random.randn(FULL_B, 3, 3, C, 1).astype(np.float32)
    out = kernel(xs, fs)
    print(out.shape, out.dtype)


# revision 12
# speedup vs baseline: 161.4993x; 1.4732x over previous
"""Batch depthwise conv2d (per-sample 3x3 filters) on 8 TRN2 NeuronCores.

x: [32, 128, 128, 128] (b, h, w, ci) fp32
f: [32, 3, 3, 128, 1]  (b, kh, kw, ci, 1) fp32
out: [32, 126, 126, 128] VALID, stride 1.

Data parallel: 4 samples per core. Per sample on each core:
  1. x loaded naturally [h, (w c)] into SBUF in w-blocks.
  2. PE transposes [h, c] slices (fixed w) -> x_t [c, s] with s = w*128 + h
     (channel-major; all 9 tap shifts become free-dim offsets dw*128+dh).
  3. 9 matmuls per output chunk: lhsT = diag(f[dh,dw,:]), rhs = x_t shifted,
     accumulated in PSUM [c, s_chunk] (fp32).
  4. PE transposes back per w-col [c, h] -> [h, c], assembled into [y, (x c)]
     tiles and DMA'd out.

Compute dtype for the tap matmuls is switchable: float32r (full PE rate at
N=512, reduced-precision multiply), bfloat16 (full rate, casts on load), or
float32 (exact, 4x slower).
"""

import os
from contextlib import ExitStack

import numpy as np

import concourse.bacc as bacc
import concourse.bass as bass
import concourse.bass_utils as _bu
import concourse.mybir as mybir
import concourse.tile as tile
from concourse.bass_utils import run_bass_kernel_spmd

# Enable walrus LDWEIGHTS dedup: consecutive matmuls sharing a stationary
# operand then skip the redundant per-matmul weight reload (~107 ns each).
if os.environ.get("BASS_LDW_OPT", "1") == "1":
    _orig_run_command = _bu.run_command

    def _patched_run_command(argv, **kw):
        argv = [
            "--enable-ldw-opt=true" if a == "--enable-ldw-opt=false" else a
            for a in argv
        ]
        return _orig_run_command(argv, **kw)

    _bu.run_command = _patched_run_command

N_CORES = 8
B_PER = 4
H = W = C = 128
HO = WO = 126
FULL_B = 32

# tap compute mode: "f32r" | "bf16" | "f32"
KMODE = os.environ.get("BASS_KMODE", "f32r")
# benchmark-only: repeat the whole body LOOP_K times in a hardware loop
LOOP_K = int(os.environ.get("BASS_LOOP_K", "1"))
# benchmark-only phase isolation: "all" | "notaps" | "notrans"
PHASES = os.environ.get("BASS_PHASES", "all")

F32 = mybir.dt.float32
F32R = mybir.dt.float32r
BF16 = mybir.dt.bfloat16

# storage dtype for x path (DMA + transposes)
SDT = BF16 if KMODE == "bf16" else F32
# dtype of x_t and diag (the tap matmul operands); fp32r producers must
# write fp32r directly (walrus requires rounded-to-fp32r inputs).
XT_DT = {"f32r": F32R, "bf16": BF16, "f32": F32}[KMODE]

S_TOT = W * H          # 16384 flattened spatial (w-major)
S_PAD = S_TOT + 384    # room for tap offsets up to 2*128+2 past valid reads
CHUNK = 512            # psum free-dim per tap accumulation group
NCHUNK = S_TOT // CHUNK  # 32
CG = 4                 # chunks per tap group (shares diag weight loads)
WBLK = 32              # w-cols per input DMA


def _emit(ctx: ExitStack, tc: tile.TileContext, loop_k: int, phases: str):
    nc = tc.nc
    x = nc.dram_tensor("x", [B_PER, H, W, C], F32, kind="ExternalInput").ap()
    f = nc.dram_tensor("f", [B_PER, 3, 3, C, 1], F32, kind="ExternalInput").ap()
    y = nc.dram_tensor("y", [B_PER, HO, WO, C], F32, kind="ExternalOutput").ap()

    xin = ctx.enter_context(tc.tile_pool(name="xin", bufs=3))
    xtp = ctx.enter_context(tc.tile_pool(name="xtp", bufs=1))
    qp = ctx.enter_context(tc.tile_pool(name="qp", bufs=4))
    outp = ctx.enter_context(tc.tile_pool(name="outp", bufs=2))
    fpool = ctx.enter_context(tc.tile_pool(name="fpool", bufs=2))
    consts = ctx.enter_context(tc.tile_pool(name="consts", bufs=1))
    ps_in = ctx.enter_context(tc.tile_pool(name="ps_in", bufs=2, space="PSUM"))
    ps_tap = ctx.enter_context(tc.tile_pool(name="ps_tap", bufs=CG, space="PSUM"))
    ps_out = ctx.enter_context(tc.tile_pool(name="ps_out", bufs=2, space="PSUM"))

    # Identity matrices. Built on gpsimd, then re-produced by ACT so that
    # every PE-side input has ACT as its single producer engine (each PE
    # transpose instruction has exactly ONE hardware sync-wait slot).
    ident_g = consts.tile([128, 128], F32)
    nc.gpsimd.memset(ident_g[:], 1.0)
    nc.gpsimd.affine_select(
        ident_g[:], ident_g[:], pattern=[[-1, 128]],
        compare_op=mybir.AluOpType.is_equal, fill=0.0, base=0,
        channel_multiplier=1,
    )
    ident_f32 = consts.tile([128, 128], F32)
    nc.scalar.copy(ident_f32[:], ident_g[:])
    if SDT == F32:
        ident_s = ident_f32
    else:
        ident_s = consts.tile([128, 128], SDT)
        nc.scalar.copy(ident_s[:], ident_g[:])

    # Dummy transpose: absorbs the one-time ACT(ident) wait so the first
    # real input transpose only needs its DMA wait.
    warm = ps_in.tile([C, CHUNK], F32, name="tp", tag="tp")
    nc.tensor.transpose(warm[:, 0:128], ident_f32[:], ident_f32[:])

    if loop_k > 1:
        loop_cm = tc.For_i(0, loop_k, 1)
        loop_cm.__enter__()

    for b in range(B_PER):
        # ---- filters: strided DMA -> [c, 9] columns -> 9 diag matrices ----
        fcols = fpool.tile([C, 9], F32)
        nc.gpsimd.dma_start(fcols[:], f[b].rearrange("kh kw c m -> c (kh kw m)"))
        diag = fpool.tile([C, 9 * C], XT_DT)
        for r in range(9):
            nc.vector.tensor_scalar_mul(
                diag[:, r * C:(r + 1) * C], ident_s[:], fcols[:, r:r + 1]
            )

        # ---- load x + transpose to channel-major x_t[c, w*128+h] ----
        # Split into two halves so sample b+1's copies into half A overlap
        # sample b's tap matmuls on half B (x_t is single-buffered).
        # A holds w-cols 0..65 (tap chunks 0..15 read w<=65);
        # B holds w-cols 64..127 plus tail pad (chunks 16..31, local offset).
        xtA = xtp.tile([C, 66 * H + 384], XT_DT, name="xtA", tag="xtA")
        xtB = xtp.tile([C, 64 * H + 384], XT_DT, name="xtB", tag="xtB")
        for wb in range(W // WBLK):
            xs = xin.tile([H, WBLK * C], SDT)
            src = x[b, :, wb * WBLK:(wb + 1) * WBLK, :].rearrange("h w c -> h (w c)")
            if SDT == F32:
                nc.sync.dma_start(xs[:], src)
            else:
                nc.gpsimd.dma_start(xs[:], src)  # SWDGE cast fp32->bf16
            for g in range(WBLK // 4):
                tp = ps_in.tile([C, 4 * H], SDT, name="tp", tag="tp")
                nj = 1 if phases == "notrans" else 4
                for j in range(nj):
                    wl = g * 4 + j
                    nc.tensor.transpose(
                        tp[:, j * H:(j + 1) * H],
                        xs[:, wl * C:(wl + 1) * C],
                        ident_s[:],
                    )
                wg = wb * WBLK + g * 4
                if wg + 4 <= 66:
                    nc.scalar.copy(xtA[:, wg * H:(wg + 4) * H], tp[:])
                elif wg >= 66:
                    nc.scalar.copy(xtB[:, (wg - 64) * H:(wg - 60) * H], tp[:])
                else:  # wg == 64: w64..67 straddles -> copy into both halves
                    nc.scalar.copy(xtA[:, 64 * H:66 * H], tp[:, 0:2 * H])
                    nc.scalar.copy(xtB[:, 0:4 * H], tp[:])

        # ---- 9-tap diag matmuls, PSUM accumulate, chunk groups of CG ----
        qs = []
        cg0 = 0
        while cg0 < NCHUNK:
            ncg = min(CG, NCHUNK - cg0)
            psums = [
                ps_tap.tile([C, CHUNK], F32, name="tap_ps", tag="tap_ps")
                for _ in range(ncg)
            ]
            ti = 0
            if phases == "notaps":
                # single tap pass keeps downstream structure with 1/9 PE tap work
                for j in range(ncg):
                    s0 = (cg0 + j) * CHUNK
                    xsrc = (xtA if cg0 + j < 16 else xtB)[:, 0:CHUNK]
                    nc.tensor.matmul(psums[j][:], diag[:, 0:C], xsrc,
                                     start=True, stop=True)
            for dw in range(phases != "notaps" and 3 or 0):
                for dh in range(3):
                    off = dw * H + dh
                    r = dh * 3 + dw  # f row index (kh=dh, kw=dw)
                    dg = diag[:, r * C:(r + 1) * C]
                    for j in range(ncg):
                        s0 = (cg0 + j) * CHUNK
                        if cg0 + j < 16:
                            xsrc = xtA[:, s0 + off: s0 + off + CHUNK]
                        else:
                            sl = s0 - 64 * H
                            xsrc = xtB[:, sl + off: sl + off + CHUNK]
                        nc.tensor.matmul(
                            psums[j][:],
                            dg,
                            xsrc,
                            start=(ti == 0),
                            stop=(ti == 8),
                        )
                    ti += 1
            for j in range(ncg):
                q = qp.tile([C, CHUNK], F32)
                nc.vector.tensor_copy(q[:], psums[j][:])
                qs.append(q)
            cg0 += ncg

        # ---- transpose back [c, h] -> [h, c] per valid w-col; DMA out ----
        for ob in range(8):  # output blocks of 16 w-cols
            w0 = ob * 16
            nw = min(16, WO - w0)  # 16,...,16,14
            osb = outp.tile([128, 16 * C], F32)
            for g in range(4):  # 4 chunks of 4 w-cols each
                chunk_idx = ob * 4 + g
                nv = max(0, min(4, WO - (w0 + g * 4)))
                if nv == 0:
                    continue
                ops_t = ps_out.tile([128, 4 * C], F32, name="ops_t", tag="ops_t")
                q = qs[chunk_idx]
                for j in range(nv if phases != "notrans" else 1):
                    nc.tensor.transpose(
                        ops_t[:, j * C:(j + 1) * C],
                        q[:, j * H:(j + 1) * H],
                        ident_f32[:],
                    )
                nc.vector.tensor_copy(
                    osb[0:HO, g * 4 * C:(g * 4 + nv) * C], ops_t[0:HO, 0:nv * C]
                )
            dst = y[b, :, w0:w0 + nw, :].rearrange("h w c -> h (w c)")
            nc.sync.dma_start(dst, osb[0:HO, 0:nw * C])

    if loop_k > 1:
        loop_cm.__exit__(None, None, None)


_NC_CACHE = {}


def _build(loop_k=None, phases=None):
    loop_k = LOOP_K if loop_k is None else loop_k
    phases = PHASES if phases is None else phases
    key = (KMODE, loop_k, phases)
    if key not in _NC_CACHE:
        nc = bacc.Bacc("TRN2", target_bir_lowering=False, debug=False)
        with tile.TileContext(nc) as tc, ExitStack() as ctx:
            _emit(ctx, tc, loop_k, phases)
        nc.compile()
        _NC_CACHE[key] = nc
    return _NC_CACHE[key]


def kernel(x: np.ndarray, f: np.ndarray) -> np.ndarray:
    assert x.shape == (FULL_B, H, W, C) and f.shape == (FULL_B, 3, 3, C, 1)
    x = np.ascontiguousarray(x, dtype=np.float32)
    f = np.ascontiguousarray(f, dtype=np.float32)
    nc = _build()
    in_maps = [
        {
            "x": x[i * B_PER:(i + 1) * B_PER],
            "f": f[i * B_PER:(i + 1) * B_PER],
        }
        for i in range(N_CORES)
    ]
    res = run_bass_kernel_spmd(nc, in_maps, core_ids=list(range(N_CORES)))
    return np.concatenate([r["y"] for r in res.results], axis=0)


if __name__ == "__main__":
    xs = np.random.randn(FULL_B, H, W, C).astype(np.float32)
    fs = np.random.randn(FULL_B, 3, 3, C, 1).astype(np.float32)
    out = kernel(xs, fs)
    print(out.shape, out.dtype)
